# revision 21
# baseline (speedup 1.0000x reference)
"""HardTripletLoss2 Trainium2 kernel.

Data-parallel over the N = B*C = 204800 row dimension of attributes/embeddings.
Each of 8 cores computes per-row squared pairwise distances
    rel[n] = || embeddings[n] - attributes[n] + 1e-6 ||_2^2
for its 25600-row shard (the memory-heavy part: 2 x 255 MB streamed).
The tiny (1024, 200) relations matrix is gathered to host, where the
column max/min reductions and final scalar loss are computed in numpy.

Per-tile compute is spread across three engines so none throttles the
HBM stream and the post-stream backlog stays small: subs alternate
GpSimd/DVE (2:1), Scalar squares most columns in one big activation
(plus a few columns via its square+accumulate path), DVE does the
segmented row-sum (tensor_reduce axis=X) for the rest. Tile sizes
taper at the end so the final serial chain is short.
"""

import os
import sys
import types

import numpy as np


def _ensure_ntff_hook_module():
    """bass_utils imports antenv.axon_hooks when BASS_TRACE is set; some
    images lack that module. Provide it (with the ctypes-based NTFF hook
    when available) so a traced run works and never crashes."""
    try:
        import antenv.axon_hooks  # noqa: F401

        return
    except ImportError:
        pass
    hook = None
    try:
        from trn_agent_boot.trn_boot import _ntff_profile_via_ctypes

        hook = _ntff_profile_via_ctypes("/opt/axon/libaxon_pjrt.so")
    except Exception:
        hook = None
    mod = types.ModuleType("antenv.axon_hooks")
    mod.get_axon_ntff_profile_hook = lambda: hook
    mod.set_axon_ntff_profile_hook = lambda h: None
    sys.modules["antenv.axon_hooks"] = mod


_ensure_ntff_hook_module()

import concourse.bacc as bacc
import concourse.tile as tile
from concourse import mybir
from concourse.bass_utils import run_bass_kernel_spmd

N_CORES = 8
B, C, D = 1024, 200, 312
N = B * C                      # 204800 rows
ROWS_PER_CORE = N // N_CORES   # 25600
P = 128                        # SBUF partitions
NT = ROWS_PER_CORE // P        # 200 rel columns per core
TILES = [10] * 17 + [8, 7, 6, 4, 3, 2]  # per-tile column counts (sum = NT)
assert sum(TILES) == NT
CH_MAX = max(TILES)
IO_BUFS = 4
KSC = 2          # per-tile columns handled by Scalar square+accum

MARGIN = 1.0
PD_EPS = 1e-6
DENOM_EPS = 1e-16

_NC_CACHE = None
LAST_RESULTS = None  # test.py reads .exec_time_ns after a traced run


def _build_nc():
    f32 = mybir.dt.float32
    nc = bacc.Bacc("TRN2", target_bir_lowering=False, debug=False)
    a = nc.dram_tensor("attributes", [ROWS_PER_CORE, D], f32, kind="ExternalInput")
    e = nc.dram_tensor("embeddings", [ROWS_PER_CORE, D], f32, kind="ExternalInput")
    rel = nc.dram_tensor("rel", [P, NT], f32, kind="ExternalOutput")

    with tile.TileContext(nc) as tc:
        with (
            tc.tile_pool(name="io", bufs=IO_BUFS) as io_pool,
            tc.tile_pool(name="res", bufs=1) as res_pool,
        ):
            eps_tile = res_pool.tile([P, 1], f32)
            nc.vector.memset(eps_tile, PD_EPS)
            res = res_pool.tile([P, NT], f32)

            col = 0
            for t, ch in enumerate(TILES):
                base = P * col
                rows = P * ch
                a_v = a.ap()[base : base + rows].rearrange(
                    "(p j) d -> p j d", j=ch
                )
                e_v = e.ap()[base : base + rows].rearrange(
                    "(p j) d -> p j d", j=ch
                )
                a_t = io_pool.tile([P, CH_MAX, D], f32, tag="a")
                e_t = io_pool.tile([P, CH_MAX, D], f32, tag="e")
                nc.sync.dma_start(out=a_t[:, :ch, :], in_=a_v)
                nc.sync.dma_start(out=e_t[:, :ch, :], in_=e_v)
                # diff = e - a, written over a_t; GpSimd takes 14 of the
                # first 20 tiles' subs (DVE the rest and the taper tiles,
                # where its lower per-op latency shortens the final chain)
                in_gp = t < 20 and (t * 14) // 20 != ((t + 1) * 14) // 20
                sub_eng = nc.gpsimd if in_gp else nc.vector
                sub_eng.tensor_sub(a_t[:, :ch, :], e_t[:, :ch, :], a_t[:, :ch, :])
                # last ksc columns: Scalar square+accumulate straight
                # into res (unloads DVE); the rest: one big square then a
                # segmented DVE reduce over a contiguous prefix slice
                ksc = min(KSC, ch - 1)
                kb = ch - ksc
                nc.scalar.activation(
                    out=e_t[:, :kb, :],
                    in_=a_t[:, :kb, :],
                    func=mybir.ActivationFunctionType.Square,
                    bias=eps_tile,
                    scale=1.0,
                )
                for j in range(kb, ch):
                    nc.scalar.activation(
                        out=e_t[:, j, :],
                        in_=a_t[:, j, :],
                        func=mybir.ActivationFunctionType.Square,
                        bias=eps_tile,
                        scale=1.0,
                        accum_out=res[:, col + j : col + j + 1],
                    )
                nc.vector.tensor_reduce(
                    out=res[:, col : col + kb],
                    in_=e_t[:, :kb, :],
                    axis=mybir.AxisListType.X,
                    op=mybir.AluOpType.add,
                )
                col += ch
            nc.sync.dma_start(out=rel.ap(), in_=res)
    nc.compile()
    return nc


def _get_nc():
    global _NC_CACHE
    if _NC_CACHE is None:
        _NC_CACHE = _build_nc()
    return _NC_CACHE


_RUNNER_CACHE = None


def _make_resident_runner(nc):
    """Like bass2jax.run_bass_via_pjrt's multi-core path, but stages all
    inputs on-device (device_put + block) BEFORE launching the NEFF, so no
    core executes while other cores' input uploads still stream into HBM."""
    import glob as _glob
    import tempfile

    import jax
    from jax.experimental.shard_map import shard_map
    from jax.sharding import Mesh, NamedSharding, PartitionSpec

    from concourse import bass2jax
    from concourse import bass_utils as BU

    bass2jax.install_neuronx_cc_hook()

    in_names, out_names, out_avals, out_shapes = [], [], [], []
    for alloc in nc.m.functions[0].allocations:
        if not isinstance(alloc, mybir.MemoryLocationSet):
            continue
        name = alloc.memorylocations[0].name
        if alloc.kind == "ExternalInput":
            in_names.append(name)
        elif alloc.kind == "ExternalOutput":
            out_names.append(name)
            shape = tuple(alloc.tensor_shape)
            dtype = mybir.dt.np(alloc.dtype)
            out_avals.append(jax.core.ShapedArray(shape, dtype))
            out_shapes.append((shape, dtype))
    n_params = len(in_names)
    n_outs = len(out_names)
    all_in_names = tuple(in_names) + tuple(out_names)

    def _body(*args):
        outs = bass2jax._bass_exec_p.bind(
            *args,
            out_avals=tuple(out_avals),
            in_names=all_in_names,
            out_names=tuple(out_names),
            lowering_input_output_aliases=(),
            sim_require_finite=False,
            sim_require_nnan=False,
            nc=nc,
        )
        return tuple(outs)

    devices = jax.devices()[:N_CORES]
    mesh = Mesh(np.asarray(devices), ("core",))
    spec = PartitionSpec("core")
    sharded = jax.jit(
        shard_map(
            _body,
            mesh=mesh,
            in_specs=(spec,) * (n_params + n_outs),
            out_specs=(spec,) * n_outs,
            check_rep=False,
        ),
        donate_argnums=tuple(range(n_params, n_params + n_outs)),
        keep_unused=True,
    )
    sharding = NamedSharding(mesh, spec)

    def run(in_maps, trace=False):
        per = [[np.asarray(m[n]) for n in in_names] for m in in_maps]
        concat_in = [
            np.concatenate([per[c][i] for c in range(N_CORES)], axis=0)
            for i in range(n_params)
        ]
        concat_zeros = [
            np.zeros((N_CORES * s[0], *s[1:]), dt) for s, dt in out_shapes
        ]
        dev_in = [jax.device_put(x, sharding) for x in concat_in]
        dev_zero = [jax.device_put(x, sharding) for x in concat_zeros]
        jax.block_until_ready(dev_in)
        jax.block_until_ready(dev_zero)

        profile_res = None
        if trace:
            from antenv.axon_hooks import get_axon_ntff_profile_hook

            hook = get_axon_ntff_profile_hook()
        else:
            hook = None
        if hook is not None and trace:
            import gauge.profiler

            tmpdir = tempfile.mkdtemp()
            model_indices = (
                list(range(N_CORES))
                if os.environ.get("BASS_PERFETTO_PROFILE_ALL_CORES")
                else [0]
            )
            with hook(tmpdir, model_indices):
                out_arrs = sharded(*dev_in, *dev_zero)
                jax.block_until_ready(out_arrs)
            if _glob.glob(os.path.join(tmpdir, "*_body*.ntff")):
                profile = gauge.profiler.Profile(
                    profile_path=BU.FishPath(tmpdir),
                    kernel_dev_mode=True,
                    profile_on_exit=False,
                    bass_kernel=nc.m,
                    offline_processing=True,
                    fname="*_body*",
                    metadata={},
                )
                profile_res = BU._process_ntff_profile(
                    profile, tmpdir, nc, list(range(N_CORES)),
                    model_indices if len(model_indices) > 1 else None,
                    False, {}, False,
                )
        else:
            out_arrs = sharded(*dev_in, *dev_zero)
            jax.block_until_ready(out_arrs)

        results = [
            {
                name: np.asarray(out_arrs[i]).reshape(
                    N_CORES, *out_avals[i].shape
                )[c]
                for i, name in enumerate(out_names)
            }
            for c in range(N_CORES)
        ]
        if profile_res is not None:
            return profile_res.as_bass_kernel_results(results)
        return BU.BassKernelResults(
            results=results,
            instructions_and_trace=None,
            profile_json=None,
            exec_time_ns=None,
        )

    return run


def _get_runner():
    global _RUNNER_CACHE
    if _RUNNER_CACHE is None:
        _RUNNER_CACHE = _make_resident_runner(_get_nc())
    return _RUNNER_CACHE


def _finalize(relations: np.ndarray, labels: np.ndarray) -> np.ndarray:
    """Column max/min reductions + scalar loss (f32, matching the reference)."""
    lab = labels.astype(np.int64)
    mask = np.zeros((B, C), dtype=np.float32)
    mask[np.arange(B), lab] = 1.0
    hardest_positive = (relations * mask).max(axis=0)
    max_anchor_neg = relations.max(axis=0)
    anchor_negative = relations + max_anchor_neg[None, :] * mask
    hardest_negative = anchor_negative.min(axis=0)
    tl = np.maximum(
        (hardest_positive - hardest_negative + np.float32(MARGIN)).astype(np.float32),
        np.float32(0.0),
    )
    num_hard = np.float32((tl > DENOM_EPS).sum())
    loss = tl.sum(dtype=np.float32) / (num_hard + np.float32(DENOM_EPS))
    return np.asarray(loss, dtype=np.float32)


def kernel(**inputs: np.ndarray) -> np.ndarray:
    global LAST_RESULTS
    attributes = np.ascontiguousarray(np.asarray(inputs["attributes"], np.float32))
    embeddings = np.ascontiguousarray(np.asarray(inputs["embeddings"], np.float32))
    labels = np.asarray(inputs["labels"])
    assert attributes.shape == (N, D) and embeddings.shape == (N, D)

    in_maps = []
    for k in range(N_CORES):
        sl = slice(k * ROWS_PER_CORE, (k + 1) * ROWS_PER_CORE)
        in_maps.append({"attributes": attributes[sl], "embeddings": embeddings[sl]})
    trace = bool(os.environ.get("BASS_TRACE")) and not os.environ.get(
        "BASS_NEVER_TRACE"
    )
    try:
        results = _get_runner()(in_maps, trace=trace)
    except Exception:
        # fall back to the stock SPMD path
        results = run_bass_kernel_spmd(
            _get_nc(), in_maps, core_ids=list(range(N_CORES))
        )
    LAST_RESULTS = results

    # rel_k[p, col+j] holds the SQUARED distance of shard row
    # 128*col + p*ch + j for tile (col, ch).
    shards = []
    for k in range(N_CORES):
        sq = results.results[k]["rel"]
        parts = []
        col = 0
        for ch in TILES:
            parts.append(sq[:, col : col + ch].reshape(-1))
            col += ch
        shards.append(np.concatenate(parts))
    relations = np.sqrt(np.concatenate(shards)).reshape(B, C)
    return _finalize(relations, labels)


# revision 22
# speedup vs baseline: 1.0207x; 1.0207x over previous
"""HardTripletLoss2 Trainium2 kernel.

Data-parallel over the N = B*C = 204800 row dimension of attributes/embeddings.
Each of 8 cores computes per-row squared pairwise distances
    rel[n] = || embeddings[n] - attributes[n] + 1e-6 ||_2^2
for its 25600-row shard (the memory-heavy part: 2 x 255 MB streamed).
The tiny (1024, 200) relations matrix is gathered to host, where the
column max/min reductions and final scalar loss are computed in numpy.

Per-tile compute is spread across three engines so none throttles the
HBM stream and the post-stream backlog stays small: subs alternate
GpSimd/DVE (2:1), Scalar squares most columns in one big activation
(plus a few columns via its square+accumulate path), DVE does the
segmented row-sum (tensor_reduce axis=X) for the rest. Tile sizes
taper at the end so the final serial chain is short.
"""

import os
import sys
import types

import numpy as np


def _ensure_ntff_hook_module():
    """bass_utils imports antenv.axon_hooks when BASS_TRACE is set; some
    images lack that module. Provide it (with the ctypes-based NTFF hook
    when available) so a traced run works and never crashes."""
    try:
        import antenv.axon_hooks  # noqa: F401

        return
    except ImportError:
        pass
    hook = None
    try:
        from trn_agent_boot.trn_boot import _ntff_profile_via_ctypes

        hook = _ntff_profile_via_ctypes("/opt/axon/libaxon_pjrt.so")
    except Exception:
        hook = None
    mod = types.ModuleType("antenv.axon_hooks")
    mod.get_axon_ntff_profile_hook = lambda: hook
    mod.set_axon_ntff_profile_hook = lambda h: None
    sys.modules["antenv.axon_hooks"] = mod


_ensure_ntff_hook_module()

import concourse.bacc as bacc
import concourse.tile as tile
from concourse import mybir
from concourse.bass_utils import run_bass_kernel_spmd

N_CORES = 8
B, C, D = 1024, 200, 312
N = B * C                      # 204800 rows
ROWS_PER_CORE = N // N_CORES   # 25600
P = 128                        # SBUF partitions
NT = ROWS_PER_CORE // P        # 200 rel columns per core
TILES = [10] * 17 + [8, 7, 6, 4, 3, 2]  # per-tile column counts (sum = NT)
assert sum(TILES) == NT
CH_MAX = max(TILES)
IO_BUFS = 4
KSC = 3          # per-tile columns handled by Scalar square+accum

MARGIN = 1.0
PD_EPS = 1e-6
DENOM_EPS = 1e-16

_NC_CACHE = None
LAST_RESULTS = None  # test.py reads .exec_time_ns after a traced run


def _build_nc():
    f32 = mybir.dt.float32
    nc = bacc.Bacc("TRN2", target_bir_lowering=False, debug=False)
    a = nc.dram_tensor("attributes", [ROWS_PER_CORE, D], f32, kind="ExternalInput")
    e = nc.dram_tensor("embeddings", [ROWS_PER_CORE, D], f32, kind="ExternalInput")
    rel = nc.dram_tensor("rel", [P, NT], f32, kind="ExternalOutput")

    with tile.TileContext(nc) as tc:
        with (
            tc.tile_pool(name="io", bufs=IO_BUFS) as io_pool,
            tc.tile_pool(name="res", bufs=1) as res_pool,
        ):
            eps_tile = res_pool.tile([P, 1], f32)
            nc.vector.memset(eps_tile, PD_EPS)
            res = res_pool.tile([P, NT], f32)

            col = 0
            for t, ch in enumerate(TILES):
                base = P * col
                rows = P * ch
                a_v = a.ap()[base : base + rows].rearrange(
                    "(p j) d -> p j d", j=ch
                )
                e_v = e.ap()[base : base + rows].rearrange(
                    "(p j) d -> p j d", j=ch
                )
                a_t = io_pool.tile([P, CH_MAX, D], f32, tag="a")
                e_t = io_pool.tile([P, CH_MAX, D], f32, tag="e")
                nc.sync.dma_start(out=a_t[:, :ch, :], in_=a_v)
                nc.sync.dma_start(out=e_t[:, :ch, :], in_=e_v)
                # diff = e - a, written over a_t; GpSimd takes 14 of the
                # first 20 tiles' subs (DVE the rest and the taper tiles,
                # where its lower per-op latency shortens the final chain)
                in_gp = t < 20 and (t * 14) // 20 != ((t + 1) * 14) // 20
                sub_eng = nc.gpsimd if in_gp else nc.vector
                sub_eng.tensor_sub(a_t[:, :ch, :], e_t[:, :ch, :], a_t[:, :ch, :])
                # last ksc columns: Scalar square+accumulate straight
                # into res (unloads DVE); the rest: one big square then a
                # segmented DVE reduce over a contiguous prefix slice
                ksc = min(KSC, ch - 1)
                kb = ch - ksc
                nc.scalar.activation(
                    out=e_t[:, :kb, :],
                    in_=a_t[:, :kb, :],
                    func=mybir.ActivationFunctionType.Square,
                    bias=eps_tile,
                    scale=1.0,
                )
                for j in range(kb, ch):
                    nc.scalar.activation(
                        out=e_t[:, j, :],
                        in_=a_t[:, j, :],
                        func=mybir.ActivationFunctionType.Square,
                        bias=eps_tile,
                        scale=1.0,
                        accum_out=res[:, col + j : col + j + 1],
                    )
                nc.vector.tensor_reduce(
                    out=res[:, col : col + kb],
                    in_=e_t[:, :kb, :],
                    axis=mybir.AxisListType.X,
                    op=mybir.AluOpType.add,
                )
                col += ch
            nc.sync.dma_start(out=rel.ap(), in_=res)
    nc.compile()
    return nc


def _get_nc():
    global _NC_CACHE
    if _NC_CACHE is None:
        _NC_CACHE = _build_nc()
    return _NC_CACHE


_RUNNER_CACHE = None


def _make_resident_runner(nc):
    """Like bass2jax.run_bass_via_pjrt's multi-core path, but stages all
    inputs on-device (device_put + block) BEFORE launching the NEFF, so no
    core executes while other cores' input uploads still stream into HBM."""
    import glob as _glob
    import tempfile

    import jax
    from jax.experimental.shard_map import shard_map
    from jax.sharding import Mesh, NamedSharding, PartitionSpec

    from concourse import bass2jax
    from concourse import bass_utils as BU

    bass2jax.install_neuronx_cc_hook()

    in_names, out_names, out_avals, out_shapes = [], [], [], []
    for alloc in nc.m.functions[0].allocations:
        if not isinstance(alloc, mybir.MemoryLocationSet):
            continue
        name = alloc.memorylocations[0].name
        if alloc.kind == "ExternalInput":
            in_names.append(name)
        elif alloc.kind == "ExternalOutput":
            out_names.append(name)
            shape = tuple(alloc.tensor_shape)
            dtype = mybir.dt.np(alloc.dtype)
            out_avals.append(jax.core.ShapedArray(shape, dtype))
            out_shapes.append((shape, dtype))
    n_params = len(in_names)
    n_outs = len(out_names)
    all_in_names = tuple(in_names) + tuple(out_names)

    def _body(*args):
        outs = bass2jax._bass_exec_p.bind(
            *args,
            out_avals=tuple(out_avals),
            in_names=all_in_names,
            out_names=tuple(out_names),
            lowering_input_output_aliases=(),
            sim_require_finite=False,
            sim_require_nnan=False,
            nc=nc,
        )
        return tuple(outs)

    devices = jax.devices()[:N_CORES]
    mesh = Mesh(np.asarray(devices), ("core",))
    spec = PartitionSpec("core")
    sharded = jax.jit(
        shard_map(
            _body,
            mesh=mesh,
            in_specs=(spec,) * (n_params + n_outs),
            out_specs=(spec,) * n_outs,
            check_rep=False,
        ),
        donate_argnums=tuple(range(n_params, n_params + n_outs)),
        keep_unused=True,
    )
    sharding = NamedSharding(mesh, spec)

    def run(in_maps, trace=False):
        per = [[np.asarray(m[n]) for n in in_names] for m in in_maps]
        concat_in = [
            np.concatenate([per[c][i] for c in range(N_CORES)], axis=0)
            for i in range(n_params)
        ]
        concat_zeros = [
            np.zeros((N_CORES * s[0], *s[1:]), dt) for s, dt in out_shapes
        ]
        dev_in = [jax.device_put(x, sharding) for x in concat_in]
        dev_zero = [jax.device_put(x, sharding) for x in concat_zeros]
        jax.block_until_ready(dev_in)
        jax.block_until_ready(dev_zero)

        profile_res = None
        if trace:
            from antenv.axon_hooks import get_axon_ntff_profile_hook

            hook = get_axon_ntff_profile_hook()
        else:
            hook = None
        if hook is not None and trace:
            import gauge.profiler

            tmpdir = tempfile.mkdtemp()
            model_indices = (
                list(range(N_CORES))
                if os.environ.get("BASS_PERFETTO_PROFILE_ALL_CORES")
                else [0]
            )
            with hook(tmpdir, model_indices):
                out_arrs = sharded(*dev_in, *dev_zero)
                jax.block_until_ready(out_arrs)
            if _glob.glob(os.path.join(tmpdir, "*_body*.ntff")):
                profile = gauge.profiler.Profile(
                    profile_path=BU.FishPath(tmpdir),
                    kernel_dev_mode=True,
                    profile_on_exit=False,
                    bass_kernel=nc.m,
                    offline_processing=True,
                    fname="*_body*",
                    metadata={},
                )
                profile_res = BU._process_ntff_profile(
                    profile, tmpdir, nc, list(range(N_CORES)),
                    model_indices if len(model_indices) > 1 else None,
                    False, {}, False,
                )
        else:
            out_arrs = sharded(*dev_in, *dev_zero)
            jax.block_until_ready(out_arrs)

        results = [
            {
                name: np.asarray(out_arrs[i]).reshape(
                    N_CORES, *out_avals[i].shape
                )[c]
                for i, name in enumerate(out_names)
            }
            for c in range(N_CORES)
        ]
        if profile_res is not None:
            return profile_res.as_bass_kernel_results(results)
        return BU.BassKernelResults(
            results=results,
            instructions_and_trace=None,
            profile_json=None,
            exec_time_ns=None,
        )

    return run


def _get_runner():
    global _RUNNER_CACHE
    if _RUNNER_CACHE is None:
        _RUNNER_CACHE = _make_resident_runner(_get_nc())
    return _RUNNER_CACHE


def _finalize(relations: np.ndarray, labels: np.ndarray) -> np.ndarray:
    """Column max/min reductions + scalar loss (f32, matching the reference)."""
    lab = labels.astype(np.int64)
    mask = np.zeros((B, C), dtype=np.float32)
    mask[np.arange(B), lab] = 1.0
    hardest_positive = (relations * mask).max(axis=0)
    max_anchor_neg = relations.max(axis=0)
    anchor_negative = relations + max_anchor_neg[None, :] * mask
    hardest_negative = anchor_negative.min(axis=0)
    tl = np.maximum(
        (hardest_positive - hardest_negative + np.float32(MARGIN)).astype(np.float32),
        np.float32(0.0),
    )
    num_hard = np.float32((tl > DENOM_EPS).sum())
    loss = tl.sum(dtype=np.float32) / (num_hard + np.float32(DENOM_EPS))
    return np.asarray(loss, dtype=np.float32)


def kernel(**inputs: np.ndarray) -> np.ndarray:
    global LAST_RESULTS
    attributes = np.ascontiguousarray(np.asarray(inputs["attributes"], np.float32))
    embeddings = np.ascontiguousarray(np.asarray(inputs["embeddings"], np.float32))
    labels = np.asarray(inputs["labels"])
    assert attributes.shape == (N, D) and embeddings.shape == (N, D)

    in_maps = []
    for k in range(N_CORES):
        sl = slice(k * ROWS_PER_CORE, (k + 1) * ROWS_PER_CORE)
        in_maps.append({"attributes": attributes[sl], "embeddings": embeddings[sl]})
    trace = bool(os.environ.get("BASS_TRACE")) and not os.environ.get(
        "BASS_NEVER_TRACE"
    )
    try:
        results = _get_runner()(in_maps, trace=trace)
    except Exception:
        # fall back to the stock SPMD path
        results = run_bass_kernel_spmd(
            _get_nc(), in_maps, core_ids=list(range(N_CORES))
        )
    LAST_RESULTS = results

    # rel_k[p, col+j] holds the SQUARED distance of shard row
    # 128*col + p*ch + j for tile (col, ch).
    shards = []
    for k in range(N_CORES):
        sq = results.results[k]["rel"]
        parts = []
        col = 0
        for ch in TILES:
            parts.append(sq[:, col : col + ch].reshape(-1))
            col += ch
        shards.append(np.concatenate(parts))
    relations = np.sqrt(np.concatenate(shards)).reshape(B, C)
    return _finalize(relations, labels)


# revision 23
# speedup vs baseline: 1.2186x; 1.1939x over previous
"""HardTripletLoss2 Trainium2 kernel.

Data-parallel over the N = B*C = 204800 row dimension of attributes/embeddings.
Each of 8 cores computes per-row squared pairwise distances
    rel[n] = || embeddings[n] - attributes[n] + 1e-6 ||_2^2
for its 25600-row shard (the memory-heavy part: 2 x 255 MB streamed).
The tiny (1024, 200) relations matrix is gathered to host, where the
column max/min reductions and final scalar loss are computed in numpy.

Per-tile compute is spread across three engines so none throttles the
HBM stream and the post-stream backlog stays small: GpSimd takes 14 of
the first 20 tiles' subs (DVE the rest), Scalar squares most columns
in one big activation plus the last KSC columns via its
square+accumulate path, and DVE row-sums the remaining columns with a
contiguous-prefix tensor_reduce (axis=X). Tile sizes taper at the end
so the final serial chain is short, and IO_BUFS=4 paces the stream to
compute so the 8 cores stay under the device-level HBM ceiling.
"""

import os
import sys
import types

import numpy as np


def _ensure_ntff_hook_module():
    """bass_utils imports antenv.axon_hooks when BASS_TRACE is set; some
    images lack that module. Provide it (with the ctypes-based NTFF hook
    when available) so a traced run works and never crashes."""
    try:
        import antenv.axon_hooks  # noqa: F401

        return
    except ImportError:
        pass
    hook = None
    try:
        from trn_agent_boot.trn_boot import _ntff_profile_via_ctypes

        hook = _ntff_profile_via_ctypes("/opt/axon/libaxon_pjrt.so")
    except Exception:
        hook = None
    mod = types.ModuleType("antenv.axon_hooks")
    mod.get_axon_ntff_profile_hook = lambda: hook
    mod.set_axon_ntff_profile_hook = lambda h: None
    sys.modules["antenv.axon_hooks"] = mod


_ensure_ntff_hook_module()

import concourse.bacc as bacc
import concourse.tile as tile
from concourse import mybir
from concourse.bass_utils import run_bass_kernel_spmd

N_CORES = 8
B, C, D = 1024, 200, 312
N = B * C                      # 204800 rows
ROWS_PER_CORE = N // N_CORES   # 25600
P = 128                        # SBUF partitions
NT = ROWS_PER_CORE // P        # 200 rel columns per core
TILES = [10] * 17 + [8, 7, 6, 4, 3, 2]  # per-tile column counts (sum = NT)
assert sum(TILES) == NT
CH_MAX = max(TILES)
IO_BUFS = 4
KSC = 3          # per-tile columns handled by Scalar square+accum

MARGIN = 1.0
PD_EPS = 1e-6
DENOM_EPS = 1e-16

_NC_CACHE = None
LAST_RESULTS = None  # test.py reads .exec_time_ns after a traced run


def _build_nc():
    f32 = mybir.dt.float32
    nc = bacc.Bacc("TRN2", target_bir_lowering=False, debug=False)
    a = nc.dram_tensor("attributes", [ROWS_PER_CORE, D], f32, kind="ExternalInput")
    e = nc.dram_tensor("embeddings", [ROWS_PER_CORE, D], f32, kind="ExternalInput")
    rel = nc.dram_tensor("rel", [P, NT], f32, kind="ExternalOutput")

    with tile.TileContext(nc) as tc:
        with (
            tc.tile_pool(name="io", bufs=IO_BUFS) as io_pool,
            tc.tile_pool(name="res", bufs=1) as res_pool,
        ):
            eps_tile = res_pool.tile([P, 1], f32)
            nc.vector.memset(eps_tile, PD_EPS)
            res = res_pool.tile([P, NT], f32)

            col = 0
            for t, ch in enumerate(TILES):
                base = P * col
                rows = P * ch
                a_v = a.ap()[base : base + rows].rearrange(
                    "(p j) d -> p j d", j=ch
                )
                e_v = e.ap()[base : base + rows].rearrange(
                    "(p j) d -> p j d", j=ch
                )
                a_t = io_pool.tile([P, CH_MAX, D], f32, tag="a")
                e_t = io_pool.tile([P, CH_MAX, D], f32, tag="e")
                nc.sync.dma_start(out=a_t[:, :ch, :], in_=a_v)
                nc.sync.dma_start(out=e_t[:, :ch, :], in_=e_v)
                # diff = e - a, written over a_t; GpSimd takes 14 of the
                # first 20 tiles' subs (DVE the rest and the taper tiles,
                # where its lower per-op latency shortens the final chain)
                in_gp = t < 20 and (t * 14) // 20 != ((t + 1) * 14) // 20
                sub_eng = nc.gpsimd if in_gp else nc.vector
                sub_eng.tensor_sub(a_t[:, :ch, :], e_t[:, :ch, :], a_t[:, :ch, :])
                # last ksc columns: Scalar square+accumulate straight
                # into res (unloads DVE); the rest: one big square then a
                # segmented DVE reduce over a contiguous prefix slice
                ksc = min(KSC, ch - 1)
                kb = ch - ksc
                nc.scalar.activation(
                    out=e_t[:, :kb, :],
                    in_=a_t[:, :kb, :],
                    func=mybir.ActivationFunctionType.Square,
                    bias=eps_tile,
                    scale=1.0,
                )
                for j in range(kb, ch):
                    nc.scalar.activation(
                        out=e_t[:, j, :],
                        in_=a_t[:, j, :],
                        func=mybir.ActivationFunctionType.Square,
                        bias=eps_tile,
                        scale=1.0,
                        accum_out=res[:, col + j : col + j + 1],
                    )
                nc.vector.tensor_reduce(
                    out=res[:, col : col + kb],
                    in_=e_t[:, :kb, :],
                    axis=mybir.AxisListType.X,
                    op=mybir.AluOpType.add,
                )
                col += ch
            nc.sync.dma_start(out=rel.ap(), in_=res)
    nc.compile()
    return nc


def _get_nc():
    global _NC_CACHE
    if _NC_CACHE is None:
        _NC_CACHE = _build_nc()
    return _NC_CACHE


_RUNNER_CACHE = None


def _make_resident_runner(nc):
    """Like bass2jax.run_bass_via_pjrt's multi-core path, but stages all
    inputs on-device (device_put + block) BEFORE launching the NEFF, so no
    core executes while other cores' input uploads still stream into HBM."""
    import glob as _glob
    import tempfile

    import jax
    from jax.experimental.shard_map import shard_map
    from jax.sharding import Mesh, NamedSharding, PartitionSpec

    from concourse import bass2jax
    from concourse import bass_utils as BU

    bass2jax.install_neuronx_cc_hook()

    in_names, out_names, out_avals, out_shapes = [], [], [], []
    for alloc in nc.m.functions[0].allocations:
        if not isinstance(alloc, mybir.MemoryLocationSet):
            continue
        name = alloc.memorylocations[0].name
        if alloc.kind == "ExternalInput":
            in_names.append(name)
        elif alloc.kind == "ExternalOutput":
            out_names.append(name)
            shape = tuple(alloc.tensor_shape)
            dtype = mybir.dt.np(alloc.dtype)
            out_avals.append(jax.core.ShapedArray(shape, dtype))
            out_shapes.append((shape, dtype))
    n_params = len(in_names)
    n_outs = len(out_names)
    all_in_names = tuple(in_names) + tuple(out_names)

    def _body(*args):
        outs = bass2jax._bass_exec_p.bind(
            *args,
            out_avals=tuple(out_avals),
            in_names=all_in_names,
            out_names=tuple(out_names),
            lowering_input_output_aliases=(),
            sim_require_finite=False,
            sim_require_nnan=False,
            nc=nc,
        )
        return tuple(outs)

    devices = jax.devices()[:N_CORES]
    mesh = Mesh(np.asarray(devices), ("core",))
    spec = PartitionSpec("core")
    sharded = jax.jit(
        shard_map(
            _body,
            mesh=mesh,
            in_specs=(spec,) * (n_params + n_outs),
            out_specs=(spec,) * n_outs,
            check_rep=False,
        ),
        donate_argnums=tuple(range(n_params, n_params + n_outs)),
        keep_unused=True,
    )
    sharding = NamedSharding(mesh, spec)

    def run(in_maps, trace=False):
        per = [[np.asarray(m[n]) for n in in_names] for m in in_maps]
        concat_in = [
            np.concatenate([per[c][i] for c in range(N_CORES)], axis=0)
            for i in range(n_params)
        ]
        concat_zeros = [
            np.zeros((N_CORES * s[0], *s[1:]), dt) for s, dt in out_shapes
        ]
        dev_in = [jax.device_put(x, sharding) for x in concat_in]
        dev_zero = [jax.device_put(x, sharding) for x in concat_zeros]
        jax.block_until_ready(dev_in)
        jax.block_until_ready(dev_zero)

        profile_res = None
        if trace:
            from antenv.axon_hooks import get_axon_ntff_profile_hook

            hook = get_axon_ntff_profile_hook()
        else:
            hook = None
        if hook is not None and trace:
            import gauge.profiler

            tmpdir = tempfile.mkdtemp()
            model_indices = (
                list(range(N_CORES))
                if os.environ.get("BASS_PERFETTO_PROFILE_ALL_CORES")
                else [0]
            )
            with hook(tmpdir, model_indices):
                out_arrs = sharded(*dev_in, *dev_zero)
                jax.block_until_ready(out_arrs)
            if _glob.glob(os.path.join(tmpdir, "*_body*.ntff")):
                profile = gauge.profiler.Profile(
                    profile_path=BU.FishPath(tmpdir),
                    kernel_dev_mode=True,
                    profile_on_exit=False,
                    bass_kernel=nc.m,
                    offline_processing=True,
                    fname="*_body*",
                    metadata={},
                )
                profile_res = BU._process_ntff_profile(
                    profile, tmpdir, nc, list(range(N_CORES)),
                    model_indices if len(model_indices) > 1 else None,
                    False, {}, False,
                )
        else:
            out_arrs = sharded(*dev_in, *dev_zero)
            jax.block_until_ready(out_arrs)

        results = [
            {
                name: np.asarray(out_arrs[i]).reshape(
                    N_CORES, *out_avals[i].shape
                )[c]
                for i, name in enumerate(out_names)
            }
            for c in range(N_CORES)
        ]
        if profile_res is not None:
            return profile_res.as_bass_kernel_results(results)
        return BU.BassKernelResults(
            results=results,
            instructions_and_trace=None,
            profile_json=None,
            exec_time_ns=None,
        )

    return run


def _get_runner():
    global _RUNNER_CACHE
    if _RUNNER_CACHE is None:
        _RUNNER_CACHE = _make_resident_runner(_get_nc())
    return _RUNNER_CACHE


def _finalize(relations: np.ndarray, labels: np.ndarray) -> np.ndarray:
    """Column max/min reductions + scalar loss (f32, matching the reference)."""
    lab = labels.astype(np.int64)
    mask = np.zeros((B, C), dtype=np.float32)
    mask[np.arange(B), lab] = 1.0
    hardest_positive = (relations * mask).max(axis=0)
    max_anchor_neg = relations.max(axis=0)
    anchor_negative = relations + max_anchor_neg[None, :] * mask
    hardest_negative = anchor_negative.min(axis=0)
    tl = np.maximum(
        (hardest_positive - hardest_negative + np.float32(MARGIN)).astype(np.float32),
        np.float32(0.0),
    )
    num_hard = np.float32((tl > DENOM_EPS).sum())
    loss = tl.sum(dtype=np.float32) / (num_hard + np.float32(DENOM_EPS))
    return np.asarray(loss, dtype=np.float32)


def kernel(**inputs: np.ndarray) -> np.ndarray:
    global LAST_RESULTS
    attributes = np.ascontiguousarray(np.asarray(inputs["attributes"], np.float32))
    embeddings = np.ascontiguousarray(np.asarray(inputs["embeddings"], np.float32))
    labels = np.asarray(inputs["labels"])
    assert attributes.shape == (N, D) and embeddings.shape == (N, D)

    in_maps = []
    for k in range(N_CORES):
        sl = slice(k * ROWS_PER_CORE, (k + 1) * ROWS_PER_CORE)
        in_maps.append({"attributes": attributes[sl], "embeddings": embeddings[sl]})
    trace = bool(os.environ.get("BASS_TRACE")) and not os.environ.get(
        "BASS_NEVER_TRACE"
    )
    try:
        results = _get_runner()(in_maps, trace=trace)
    except Exception:
        # fall back to the stock SPMD path
        results = run_bass_kernel_spmd(
            _get_nc(), in_maps, core_ids=list(range(N_CORES))
        )
    LAST_RESULTS = results

    # rel_k[p, col+j] holds the SQUARED distance of shard row
    # 128*col + p*ch + j for tile (col, ch).
    shards = []
    for k in range(N_CORES):
        sq = results.results[k]["rel"]
        parts = []
        col = 0
        for ch in TILES:
            parts.append(sq[:, col : col + ch].reshape(-1))
            col += ch
        shards.append(np.concatenate(parts))
    relations = np.sqrt(np.concatenate(shards)).reshape(B, C)
    return _finalize(relations, labels)
